# revision 3
# baseline (speedup 1.0000x reference)
"""Trainium2 Bass kernel for nn_MultiHeadAttention_31112743092867.

Problem: B=2 batches, single query token (SQ=1), SK=32768 keys, D=1024,
16 heads (head dim 64), full projections + softmax attention + out
projection + LayerNorm. Returns (feat [2,1024], raw scores[0] [16,1,32768]).

Because SQ == 1, the enormous k/v projections (2 x 137 GFLOP) fold away:

  scores[b,h,i] = scale * qp[b,h] . (Wk_h^T k[b,i] + bk_h)
                = qeff[b,h] . k[b,i] + const[b,h]
      with qeff[b,h] = scale * Wk_h @ qp[b,h]   (tiny, host-side)

  attn_av[b,h]  = sum_i softmax_i * (Wv_h^T v[b,i] + bv_h)
                = Wv_h^T (sum_i P[b,h,i] v[b,i]) / Z[b,h] + bv_h
      with P = exp(scores_dev), Z = sum_i P   (softmax shift-invariant, so
      const[b,h] cancels; logits are ~N(0,1) so exp without max-subtraction
      is safe in fp32 -- verified max |logit| < 7.0 on the fixed seed)

The device kernel therefore only streams k and v once (memory-bound,
512 MB total / 64 MB per core across 8 cores) computing
  scoresT = k_shard @ qeff^T    and    pv = exp(scores)^T @ v_shard
All O(D^2) projection work and the final LayerNorm run on host (microseconds).

Sharding: k/v split along the key dimension, 4096 keys per core;
qeff replicated. Partial pv/Z combine by plain summation on host (no
per-shard max, so partials are directly compatible).
"""

import numpy as np
from contextlib import ExitStack

NUM_HEADS = 16
B = 2
SK = 32768
D = 1024
DH = 64  # head dim
NCORES = 8
KPC = SK // NCORES  # 4096 keys per core
CHUNK = 1024  # keys per inner iteration
NCHUNK = KPC // CHUNK  # 4
JT = CHUNK // 128  # 8 key sub-tiles per chunk
DT = D // 128  # 8 contraction tiles
SCALE = 1.0 / np.sqrt(DH)

_NC_CACHE = {}


def _build_nc(reps=1):
    """Build the per-core Bass program (same program on all 8 cores)."""
    import concourse.mybir as mybir
    import concourse.tile as tile
    from concourse import bacc

    f32 = mybir.dt.float32
    nc = bacc.Bacc()

    ktp = nc.declare_dram_parameter("ktp", [B, D, KPC], f32, isOutput=False)
    vtp = nc.declare_dram_parameter("vtp", [B, KPC, D], f32, isOutput=False)
    qet = nc.declare_dram_parameter("qet", [B, 128, DT, NUM_HEADS], f32, isOutput=False)
    sco = nc.declare_dram_parameter(
        "sco", [B, NCHUNK, 128, JT, NUM_HEADS], f32, isOutput=True
    )
    pvo = nc.declare_dram_parameter("pvo", [B, NUM_HEADS, D], f32, isOutput=True)

    # d-major transposed k:  row (dt*128+p) of ktp[b] holds k[b, :, dt*128+p]
    ktp_r = ktp.rearrange("b (dt p) n -> b p dt n", p=128)
    # natural v grouped into 128-key sub-tiles
    vtp_r = vtp.rearrange("b (c j p) d -> b c p j d", p=128, j=JT)
    qet_r = qet.rearrange("b p dt h -> p b dt h")

    with tile.TileContext(nc) as tc, ExitStack() as ctx:
        singles = ctx.enter_context(tc.tile_pool(name="singles", bufs=1))
        loads = ctx.enter_context(tc.tile_pool(name="loads", bufs=2))
        smalls = ctx.enter_context(tc.tile_pool(name="smalls", bufs=3))
        psums = ctx.enter_context(tc.tile_pool(name="psums", bufs=2, space="PSUM"))
        pvps = ctx.enter_context(tc.tile_pool(name="pvps", bufs=2, space="PSUM"))

        qe = singles.tile([128, B, DT, NUM_HEADS], f32)
        nc.sync.dma_start(out=qe, in_=qet_r)
        zbias = singles.tile([128, 1], f32)
        nc.vector.memset(zbias, 0.0)

        for _rep in range(reps):
            for b in range(B):
                pv_ps = pvps.tile([NUM_HEADS, D], f32, tag="pv_ps")
                for c in range(NCHUNK):
                    kt = loads.tile([128, DT, CHUNK], f32, tag="kt")
                    nc.sync.dma_start(
                        out=kt, in_=ktp_r[b, :, :, c * CHUNK : (c + 1) * CHUNK]
                    )
                    vt = loads.tile([128, JT, D], f32, tag="vt")
                    nc.sync.dma_start(out=vt, in_=vtp_r[b, c])

                    # scores^T [key, head] for this chunk, accumulated over d
                    st_ps = psums.tile([128, JT, NUM_HEADS], f32, tag="st_ps")
                    for j in range(JT):
                        for dt in range(DT):
                            nc.tensor.matmul(
                                st_ps[:, j, :],
                                kt[:, dt, j * 128 : (j + 1) * 128],
                                qe[:, b, dt, :],
                                start=(dt == 0),
                                stop=(dt == DT - 1),
                            )

                    # raw scores out (host re-assembles + adds const)
                    sraw = smalls.tile([128, JT, NUM_HEADS], f32, tag="sraw")
                    nc.vector.tensor_copy(out=sraw, in_=st_ps)
                    nc.sync.dma_start(out=sco[b, c], in_=sraw)

                    # P^T = exp(scores^T)
                    pt = smalls.tile([128, JT, NUM_HEADS], f32, tag="pt")
                    nc.scalar.activation(
                        out=pt,
                        in_=st_ps,
                        func=mybir.ActivationFunctionType.Exp,
                        bias=zbias,
                    )

                    # pv[h, :] += P^T[key, h]^T @ v[key, :]
                    for j in range(JT):
                        for half in range(2):
                            nc.tensor.matmul(
                                pv_ps[:, half * 512 : (half + 1) * 512],
                                pt[:, j, :],
                                vt[:, j, half * 512 : (half + 1) * 512],
                                start=(c == 0 and j == 0),
                                stop=(c == NCHUNK - 1 and j == JT - 1),
                            )

                pv_sb = smalls.tile([NUM_HEADS, D], f32, tag="pv_sb")
                nc.vector.tensor_copy(out=pv_sb, in_=pv_ps)
                nc.sync.dma_start(out=pvo[b], in_=pv_sb)

    return nc


def _get_nc(reps=1):
    if reps not in _NC_CACHE:
        nc = _build_nc(reps)
        nc.finalize()
        _NC_CACHE[reps] = nc
    return _NC_CACHE[reps]


def _host_prep(q, Wq, bq, Wk, bk):
    """qp and qeff in float64 on host (tiny)."""
    qp = q[:, 0, :].astype(np.float64) @ Wq.astype(np.float64) + bq.astype(np.float64)
    qph = qp.reshape(B, NUM_HEADS, DH)
    Wk_h = Wk.astype(np.float64).reshape(D, NUM_HEADS, DH)
    qe = np.einsum("dhe,bhe->bhd", Wk_h, qph) * SCALE  # [B, H, D]
    const = SCALE * np.einsum(
        "bhe,he->bh", qph, bk.astype(np.float64).reshape(NUM_HEADS, DH)
    )
    return qp, qe, const


def _make_in_maps(k, v, qe):
    # device layout for qeff: [b, p, dt, h] = qe[b, h, dt*128+p]
    qet = (
        qe.astype(np.float32)
        .reshape(B, NUM_HEADS, DT, 128)
        .transpose(0, 3, 2, 1)
        .copy()
    )
    in_maps = []
    for ci in range(NCORES):
        ks = k[:, ci * KPC : (ci + 1) * KPC, :]
        vs = v[:, ci * KPC : (ci + 1) * KPC, :]
        in_maps.append(
            {
                "ktp": np.ascontiguousarray(ks.transpose(0, 2, 1)),
                "vtp": np.ascontiguousarray(vs),
                "qet": qet,
            }
        )
    return in_maps


def _run_device(in_maps, reps=1):
    from concourse.bass_utils import run_bass_kernel_spmd

    nc = _get_nc(reps)
    res = run_bass_kernel_spmd(nc, in_maps, list(range(NCORES)))
    return res.results


def kernel(q, k, v, Wq, bq, Wk, bk, Wv, bv, Wo, bo, ln_w, ln_b):
    q = np.asarray(q)
    k = np.asarray(k)
    v = np.asarray(v)

    qp, qe, const = _host_prep(q, Wq, bq, Wk, bk)
    in_maps = _make_in_maps(k, v, qe)
    results = _run_device(in_maps)

    # gather: scores [B, H, SK] (device layout [b, c, p, j, h])
    scores = np.empty((B, NUM_HEADS, SK), np.float32)
    pv = np.zeros((B, NUM_HEADS, D), np.float64)
    for ci in range(NCORES):
        sc = results[ci]["sco"]  # [B, NCHUNK, 128, JT, H]
        sc = sc.transpose(0, 4, 1, 3, 2).reshape(B, NUM_HEADS, KPC)
        scores[:, :, ci * KPC : (ci + 1) * KPC] = sc
        pv += results[ci]["pvo"].astype(np.float64)

    # softmax normalizer from the raw device scores (matches device exp to ~1 ulp)
    Z = np.exp(scores.astype(np.float64)).sum(axis=-1)  # [B, H]

    # fold the v-projection: attn_out[b,h] = Wv_h^T (pv/Z) + bv_h
    Wv_h = np.asarray(Wv, np.float64).reshape(D, NUM_HEADS, DH)
    attn_out = np.einsum("dhe,bhd->bhe", Wv_h, pv / Z[..., None]) + np.asarray(
        bv, np.float64
    ).reshape(NUM_HEADS, DH)
    merged = attn_out.reshape(B, D)

    out = merged @ np.asarray(Wo, np.float64) + np.asarray(bo, np.float64)
    mu = out.mean(-1, keepdims=True)
    var = ((out - mu) ** 2).mean(-1, keepdims=True)
    out = (out - mu) / np.sqrt(var + 1e-6) * np.asarray(ln_w, np.float64) + np.asarray(
        ln_b, np.float64
    )
    feat = out.astype(np.float32)  # [B, D]

    attn = (scores + const[..., None].astype(np.float32))[0]
    attn = attn.reshape(NUM_HEADS, 1, SK).astype(np.float32)
    return feat, attn


# revision 8
# speedup vs baseline: 2061.5310x; 2061.5310x over previous
"""Trainium2 Bass kernel for nn_MultiHeadAttention_31112743092867.

Problem: B=2 batches, single query token (SQ=1), SK=32768 keys, D=1024,
16 heads (head dim 64), full projections + softmax attention + out
projection + LayerNorm. Returns (feat [2,1024], raw scores[0] [16,1,32768]).

Because SQ == 1, the enormous k/v projections (2 x 137 GFLOP) fold away:

  scores[b,h,i] = scale * qp[b,h] . (Wk_h^T k[b,i] + bk_h)
                = qeff[b,h] . k[b,i] + const[b,h]
      with qeff[b,h] = scale * Wk_h @ qp[b,h]   (tiny, host-side)

  attn_av[b,h]  = sum_i softmax_i * (Wv_h^T v[b,i] + bv_h)
                = Wv_h^T (sum_i P[b,h,i] v[b,i]) / Z[b,h] + bv_h
      with P = exp(scores_dev), Z = sum_i P   (softmax shift-invariant, so
      const[b,h] cancels; logits are ~N(0,1) so exp without max-subtraction
      is safe in fp32 -- verified max |logit| < 7.0 on the fixed seed)

The device kernel therefore only streams k and v once (memory-bound,
512 MB total / 64 MB per core across 8 cores) computing
  scoresT = k_shard @ qeff^T    and    pv = exp(scores)^T @ v_shard
All O(D^2) projection work and the final LayerNorm run on host (microseconds).

Sharding: k/v split along the key dimension, 4096 keys per core;
qeff replicated. Partial pv/Z combine by plain summation on host (no
per-shard max, so partials are directly compatible).
"""

import contextlib
import numpy as np
from contextlib import ExitStack

NUM_HEADS = 16
B = 2
SK = 32768
D = 1024
DH = 64  # head dim
NCORES = 8
KPC = SK // NCORES  # 4096 keys per core
CHUNK = 1024  # keys per inner iteration
NCHUNK = KPC // CHUNK  # 4
JT = CHUNK // 128  # 8 key sub-tiles per chunk
DT = D // 128  # 8 contraction tiles
SCALE = 1.0 / np.sqrt(DH)

_NC_CACHE = {}


def _build_nc(reps=1, hw_loop=0):
    """Build the per-core Bass program (same program on all 8 cores).

    reps: python-unrolled repetitions of the body (for delta timing).
    hw_loop: if >0, additionally wrap the body in a hardware For_i loop
             with this trip count (timing only).
    """
    import concourse.mybir as mybir
    import concourse.tile as tile
    from concourse import bacc

    f32 = mybir.dt.float32
    nc = bacc.Bacc()

    ktp = nc.declare_dram_parameter("ktp", [B, D, KPC], f32, isOutput=False)
    vtp = nc.declare_dram_parameter("vtp", [B, KPC, D], f32, isOutput=False)
    qet = nc.declare_dram_parameter("qet", [B, 128, DT, NUM_HEADS], f32, isOutput=False)
    sco = nc.declare_dram_parameter(
        "sco", [B, NCHUNK, 128, JT, NUM_HEADS], f32, isOutput=True
    )
    pvo = nc.declare_dram_parameter("pvo", [B, NUM_HEADS, D], f32, isOutput=True)

    # d-major transposed k:  row (dt*128+p) of ktp[b] holds k[b, :, dt*128+p]
    ktp_r = ktp.rearrange("b (dt p) n -> b p dt n", p=128)
    # natural v grouped into 128-key sub-tiles
    vtp_r = vtp.rearrange("b (c j p) d -> b c p j d", p=128, j=JT)
    qet_r = qet.rearrange("b p dt h -> p b dt h")

    with tile.TileContext(nc) as tc, ExitStack() as ctx:
        singles = ctx.enter_context(tc.tile_pool(name="singles", bufs=1))
        loads = ctx.enter_context(tc.tile_pool(name="loads", bufs=2))
        smalls = ctx.enter_context(tc.tile_pool(name="smalls", bufs=3))
        psums = ctx.enter_context(tc.tile_pool(name="psums", bufs=2, space="PSUM"))
        pvps = ctx.enter_context(tc.tile_pool(name="pvps", bufs=2, space="PSUM"))

        qe = singles.tile([128, B, DT, NUM_HEADS], f32)
        nc.sync.dma_start(out=qe, in_=qet_r)
        zbias = singles.tile([128, 1], f32)
        nc.vector.memset(zbias, 0.0)

        loop_cm = (
            tc.For_i(0, hw_loop, 1, hint_engines=tuple(nc.engines))
            if hw_loop > 0
            else contextlib.nullcontext()
        )
        with loop_cm:
            for _rep in range(reps):
                for b in range(B):
                    pv_ps = pvps.tile([NUM_HEADS, D], f32, tag="pv_ps")
                    for c in range(NCHUNK):
                        kt = loads.tile([128, DT, CHUNK], f32, tag="kt")
                        nc.sync.dma_start(
                            out=kt, in_=ktp_r[b, :, :, c * CHUNK : (c + 1) * CHUNK]
                        )
                        vt = loads.tile([128, JT, D], f32, tag="vt")
                        nc.sync.dma_start(out=vt, in_=vtp_r[b, c])

                        # scores^T [key, head] for this chunk, accumulated over d
                        st_ps = psums.tile([128, JT, NUM_HEADS], f32, tag="st_ps")
                        for j in range(JT):
                            for dt in range(DT):
                                nc.tensor.matmul(
                                    st_ps[:, j, :],
                                    kt[:, dt, j * 128 : (j + 1) * 128],
                                    qe[:, b, dt, :],
                                    start=(dt == 0),
                                    stop=(dt == DT - 1),
                                )

                        # raw scores out (host re-assembles + adds const)
                        sraw = smalls.tile([128, JT, NUM_HEADS], f32, tag="sraw")
                        nc.vector.tensor_copy(out=sraw, in_=st_ps)
                        nc.sync.dma_start(out=sco[b, c], in_=sraw)

                        # P^T = exp(scores^T)
                        pt = smalls.tile([128, JT, NUM_HEADS], f32, tag="pt")
                        nc.scalar.activation(
                            out=pt,
                            in_=st_ps,
                            func=mybir.ActivationFunctionType.Exp,
                            bias=zbias,
                        )

                        # pv[h, :] += P^T[key, h]^T @ v[key, :]
                        for j in range(JT):
                            for half in range(2):
                                nc.tensor.matmul(
                                    pv_ps[:, half * 512 : (half + 1) * 512],
                                    pt[:, j, :],
                                    vt[:, j, half * 512 : (half + 1) * 512],
                                    start=(c == 0 and j == 0),
                                    stop=(c == NCHUNK - 1 and j == JT - 1),
                                )

                    pv_sb = smalls.tile([NUM_HEADS, D], f32, tag="pv_sb")
                    nc.vector.tensor_copy(out=pv_sb, in_=pv_ps)
                    nc.sync.dma_start(out=pvo[b], in_=pv_sb)

    return nc


def _get_nc(reps=1, hw_loop=0):
    key = (reps, hw_loop)
    if key not in _NC_CACHE:
        nc = _build_nc(reps, hw_loop)
        nc.finalize()
        _NC_CACHE[key] = nc
    return _NC_CACHE[key]


def _host_prep(q, Wq, bq, Wk, bk):
    """qp and qeff in float64 on host (tiny)."""
    qp = q[:, 0, :].astype(np.float64) @ Wq.astype(np.float64) + bq.astype(np.float64)
    qph = qp.reshape(B, NUM_HEADS, DH)
    Wk_h = Wk.astype(np.float64).reshape(D, NUM_HEADS, DH)
    qe = np.einsum("dhe,bhe->bhd", Wk_h, qph) * SCALE  # [B, H, D]
    const = SCALE * np.einsum(
        "bhe,he->bh", qph, bk.astype(np.float64).reshape(NUM_HEADS, DH)
    )
    return qp, qe, const


def _make_in_maps(k, v, qe):
    # device layout for qeff: [b, p, dt, h] = qe[b, h, dt*128+p]
    qet = (
        qe.astype(np.float32)
        .reshape(B, NUM_HEADS, DT, 128)
        .transpose(0, 3, 2, 1)
        .copy()
    )
    in_maps = []
    for ci in range(NCORES):
        ks = k[:, ci * KPC : (ci + 1) * KPC, :]
        vs = v[:, ci * KPC : (ci + 1) * KPC, :]
        in_maps.append(
            {
                "ktp": np.ascontiguousarray(ks.transpose(0, 2, 1)),
                "vtp": np.ascontiguousarray(vs),
                "qet": qet,
            }
        )
    return in_maps


def _run_device(in_maps, reps=1):
    from concourse.bass_utils import run_bass_kernel_spmd

    nc = _get_nc(reps)
    res = run_bass_kernel_spmd(nc, in_maps, list(range(NCORES)))
    return res.results


def kernel(q, k, v, Wq, bq, Wk, bk, Wv, bv, Wo, bo, ln_w, ln_b):
    q = np.asarray(q)
    k = np.asarray(k)
    v = np.asarray(v)

    qp, qe, const = _host_prep(q, Wq, bq, Wk, bk)
    in_maps = _make_in_maps(k, v, qe)
    results = _run_device(in_maps)

    # gather: scores [B, H, SK] (device layout [b, c, p, j, h])
    scores = np.empty((B, NUM_HEADS, SK), np.float32)
    pv = np.zeros((B, NUM_HEADS, D), np.float64)
    for ci in range(NCORES):
        sc = results[ci]["sco"]  # [B, NCHUNK, 128, JT, H]
        sc = sc.transpose(0, 4, 1, 3, 2).reshape(B, NUM_HEADS, KPC)
        scores[:, :, ci * KPC : (ci + 1) * KPC] = sc
        pv += results[ci]["pvo"].astype(np.float64)

    # softmax normalizer from the raw device scores (matches device exp to ~1 ulp)
    Z = np.exp(scores.astype(np.float64)).sum(axis=-1)  # [B, H]

    # fold the v-projection: attn_out[b,h] = Wv_h^T (pv/Z) + bv_h
    Wv_h = np.asarray(Wv, np.float64).reshape(D, NUM_HEADS, DH)
    attn_out = np.einsum("dhe,bhd->bhe", Wv_h, pv / Z[..., None]) + np.asarray(
        bv, np.float64
    ).reshape(NUM_HEADS, DH)
    merged = attn_out.reshape(B, D)

    out = merged @ np.asarray(Wo, np.float64) + np.asarray(bo, np.float64)
    mu = out.mean(-1, keepdims=True)
    var = ((out - mu) ** 2).mean(-1, keepdims=True)
    out = (out - mu) / np.sqrt(var + 1e-6) * np.asarray(ln_w, np.float64) + np.asarray(
        ln_b, np.float64
    )
    feat = out.astype(np.float32)  # [B, D]

    attn = (scores + const[..., None].astype(np.float32))[0]
    attn = attn.reshape(NUM_HEADS, 1, SK).astype(np.float32)
    return feat, attn
